# revision 22
# baseline (speedup 1.0000x reference)
"""BitNetLinear on 8 Trainium2 NeuronCores.

Computes out = x @ sign(weight).T + bias for x[4,2048,4096] f32,
weight[4096,4096] f32, bias[4096] f32.

Strategy: 4-way data parallel over rows x 2-way tensor parallel over
out_features (each core owns a [2048, 2048] block of the [8192, 4096]
output; no collectives, host stitches blocks).

Per core the matmul runs in fp16 hi/lo split: x = hi + lo with both
halves fp16 (sign(weight) is exactly representable in fp16), both
passes accumulated into the same PSUM banks in fp32. This gives
~fp32 accuracy (measured max err / scale ~5e-7 vs float64) at bf16
matmul speed: 1 PE cycle per moving row vs 4 for true fp32.

Layouts are precomputed on the host so every DMA is contiguous:
  xt[mt, d, db*128+m] = x_pass[m0 + mt*128 + m, db*128 + d]
  wt[db, d, o]        = sign(weight)[o0 + o, db*128 + d]
The kernel keeps all 32 weight blocks resident in SBUF (128 KB per
partition), streams x tiles (one 1 MB contiguous DMA per pass per
m-tile), and accumulates each [128, 512] output chunk over 64 matmuls
(2 passes x 32 k-blocks) before a DVE eviction fused with the bias add.
"""

import sys
import types

import numpy as np

import concourse.mybir as mybir
import concourse.tile as tile
from concourse import bacc
from concourse.bass_utils import run_bass_kernel_spmd


def _ensure_axon_hooks():
    """run_bass_kernel_spmd(trace=True) (or BASS_TRACE=1 in the env) imports
    antenv.axon_hooks, which some agent images lack. Provide it, and register
    the ctypes NTFF hook if the boot shim is available, so tracing works (or
    degrades to a warning) instead of crashing."""
    try:
        import antenv.axon_hooks  # noqa: F401

        return
    except ImportError:
        pass
    m = types.ModuleType("antenv.axon_hooks")
    m._h = None
    m.set_axon_ntff_profile_hook = lambda h: setattr(m, "_h", h)
    m.get_axon_ntff_profile_hook = lambda: m._h
    sys.modules["antenv.axon_hooks"] = m
    try:
        import antenv

        antenv.axon_hooks = m
    except ImportError:
        pass
    try:
        from trn_agent_boot.trn_boot import _ntff_profile_via_ctypes

        m.set_axon_ntff_profile_hook(
            _ntff_profile_via_ctypes("/opt/axon/libaxon_pjrt.so")
        )
    except Exception:
        pass


_ensure_axon_hooks()

B, S, D_IN, D_OUT = 4, 2048, 4096, 4096
M_TOT = B * S  # 8192
N_CORES = 8
MG, OG = 4, 2  # data-parallel row groups x tensor-parallel out_feature groups
M_SH = M_TOT // MG  # 2048 rows per core
O_SH = D_OUT // OG  # 2048 out features per core
P = 128
DB = D_IN // P  # 32 contraction blocks
MT = M_SH // P  # 16 m-tiles per core
NF = 512  # moving free dim per matmul (one PSUM bank of fp32)
NCH = O_SH // NF  # 4 output chunks per m-tile

_CACHE = {}


def _build():
    nc = bacc.Bacc("TRN2", target_bir_lowering=False, debug=False)
    xt_hi_d = nc.dram_tensor(
        "xt_hi", [MT, P, DB * P], mybir.dt.float16, kind="ExternalInput"
    )
    xt_lo_d = nc.dram_tensor(
        "xt_lo", [MT, P, DB * P], mybir.dt.float16, kind="ExternalInput"
    )
    wt_d = nc.dram_tensor("wt", [DB, P, O_SH], mybir.dt.float16, kind="ExternalInput")
    # x for m-tiles 0..1 again, but in k-block-major layout for the startup
    # phase: contiguous [128, 128] blocks ordered (db, mt, pass).
    xt_pair_d = nc.dram_tensor(
        "xt_pair", [DB, 2, 2, P, P], mybir.dt.float16, kind="ExternalInput"
    )
    bias_d = nc.dram_tensor("biasb", [P, O_SH], mybir.dt.float32, kind="ExternalInput")
    out_d = nc.dram_tensor("out", [M_SH, O_SH], mybir.dt.float32, kind="ExternalOutput")

    with tile.TileContext(nc) as tc:
        with (
            tc.tile_pool(name="wpool", bufs=1) as wpool,
            tc.tile_pool(name="xpool", bufs=2) as xpool,
            tc.tile_pool(name="psum", bufs=2, space="PSUM") as psum_pool,
        ):

            def load_x(mt):
                x_hi = xpool.tile(
                    [P, DB * P], mybir.dt.float16, name="x_hi", tag="xhi"
                )
                x_lo = xpool.tile(
                    [P, DB * P], mybir.dt.float16, name="x_lo", tag="xlo"
                )
                nc.sync.dma_start(out=x_hi[:], in_=xt_hi_d[mt])
                nc.sync.dma_start(out=x_lo[:], in_=xt_lo_d[mt])
                return x_hi, x_lo

            def alloc_psums(mt):
                return [
                    psum_pool.tile(
                        [P, NF], mybir.dt.float32, name=f"ps{oc}", tag=f"ps{oc}"
                    )
                    for oc in range(NCH)
                ]

            def mm_block(x_pair, psums, db, first, last):
                # 8 matmuls: hi+lo pass over one k-block into 4 psum banks
                for i, x_sb in enumerate(x_pair):
                    lhsT = x_sb[:, db * P : (db + 1) * P]
                    for oc in range(NCH):
                        nc.tensor.matmul(
                            psums[oc][:],
                            lhsT,
                            w_sb[db][:, oc * NF : (oc + 1) * NF],
                            start=first and i == 0,
                            stop=last and i == 1,
                        )

            def evict(opool, mt, psums):
                for oc in range(NCH):
                    o_sb = opool.tile(
                        [P, NF], mybir.dt.float32, name="o_sb", tag=f"o{oc}"
                    )
                    nc.vector.tensor_add(
                        o_sb[:], psums[oc][:], bias_sb[:, oc * NF : (oc + 1) * NF]
                    )
                    nc.sync.dma_start(
                        out=out_d[mt * P : (mt + 1) * P, oc * NF : (oc + 1) * NF],
                        in_=o_sb[:],
                    )

            w_sb = []
            with tc.tile_pool(name="xstart", bufs=1) as xstart_pool:
                # Startup phase x: per-k-block [128, 128] tiles so the first
                # matmul only waits for ~1.1 MB (w0 + 4 small x blocks), not
                # whole 2 MB x tiles behind the weight stream.
                xs_sb = {}
                for db in range(DB):
                    for mt in range(2):
                        for pi in range(2):
                            t = xstart_pool.tile(
                                [P, P],
                                mybir.dt.float16,
                                name=f"xs{db}_{mt}_{pi}",
                                tag=f"xs{db}_{mt}_{pi}",
                            )
                            nc.sync.dma_start(out=t[:], in_=xt_pair_d[db, mt, pi])
                            xs_sb[db, mt, pi] = t
                    # interleave weight blocks with startup-x so w[db] arrives
                    # roughly when the PE needs it
                    w = wpool.tile(
                        [P, O_SH], mybir.dt.float16, name=f"w{db}", tag=f"w{db}"
                    )
                    if db == 0:
                        # split w0 into partition chunks (each still fully
                        # contiguous in DRAM) so the eight DMA queues finish
                        # the first block together and the PE starts early
                        for j in range(4):
                            nc.sync.dma_start(
                                out=w[j * 32 : (j + 1) * 32, :],
                                in_=wt_d[db, j * 32 : (j + 1) * 32, :],
                            )
                    else:
                        nc.sync.dma_start(out=w[:], in_=wt_d[db])
                    w_sb.append(w)
                bias_sb = wpool.tile([P, O_SH], mybir.dt.float32, name="bias_sb")
                nc.sync.dma_start(out=bias_sb[:], in_=bias_d[:])

                # Prefetch x for m-tiles 2..3 now: their dma_starts must sit
                # ahead of the pair-phase eviction DMAs in the in-order sync
                # stream, else they head-of-line block until the pair phase
                # fully drains.
                x_next = {mt: load_x(mt) for mt in (2, 3)}

                # Startup phase: m-tiles 0+1 jointly, k-block-major, so PE
                # consumption (~3.4us per k-block) paces the weight stream
                # (~3.1us per block) instead of stalling for the full preload.
                psums01 = [alloc_psums(0), alloc_psums(1)]
                for db in range(DB):
                    for mt in range(2):
                        for pi in range(2):
                            lhsT = xs_sb[db, mt, pi]
                            for oc in range(NCH):
                                nc.tensor.matmul(
                                    psums01[mt][oc][:],
                                    lhsT[:],
                                    w_sb[db][:, oc * NF : (oc + 1) * NF],
                                    start=db == 0 and pi == 0,
                                    stop=db == DB - 1 and pi == 1,
                                )
            # opool created only after xstart is released so their SBUF
            # address ranges can overlap in time.
            with tc.tile_pool(name="opool", bufs=2) as opool:
                for mt in range(2):
                    evict(opool, mt, psums01[mt])

                # Steady state: one m-tile at a time, double-buffered.
                for mt in range(2, MT):
                    x_pair = x_next.pop(mt) if mt in x_next else load_x(mt)
                    psums = alloc_psums(mt)
                    if mt < MT - 1:
                        for db in range(DB):
                            mm_block(x_pair, psums, db, db == 0, db == DB - 1)
                        evict(opool, mt, psums)
                    else:
                        # Last m-tile: oc-major so each output chunk's 64-matmul
                        # accumulation finishes (and evicts) as early as
                        # possible instead of all four at the very end.
                        for oc in range(NCH):
                            for db in range(DB):
                                for i, x_sb in enumerate(x_pair):
                                    nc.tensor.matmul(
                                        psums[oc][:],
                                        x_sb[:, db * P : (db + 1) * P],
                                        w_sb[db][:, oc * NF : (oc + 1) * NF],
                                        start=db == 0 and i == 0,
                                        stop=db == DB - 1 and i == 1,
                                    )
                            o_sb = opool.tile(
                                [P, NF], mybir.dt.float32, name="o_sb", tag=f"o{oc}"
                            )
                            nc.vector.tensor_add(
                                o_sb[:],
                                psums[oc][:],
                                bias_sb[:, oc * NF : (oc + 1) * NF],
                            )
                            nc.sync.dma_start(
                                out=out_d[
                                    mt * P : (mt + 1) * P, oc * NF : (oc + 1) * NF
                                ],
                                in_=o_sb[:],
                            )
    nc.compile()
    return nc


def _prep_inputs(x, weight, bias):
    x = np.asarray(x, dtype=np.float32)
    weight = np.asarray(weight, dtype=np.float32)
    bias = np.asarray(bias, dtype=np.float32)
    xf = np.ascontiguousarray(x.reshape(M_TOT, D_IN), dtype=np.float32)
    x_hi = xf.astype(np.float16)
    x_lo = (xf - x_hi.astype(np.float32)).astype(np.float16)

    qw = np.sign(weight.astype(np.float32)).astype(np.float16)  # [o, d]

    # per o-group weight block + broadcast bias, shared by all cores in group
    wt_og, bias_og = [], []
    for og in range(OG):
        o0 = og * O_SH
        blk = np.ascontiguousarray(qw[o0 : o0 + O_SH, :].T)  # [d, o]
        wt_og.append(blk.reshape(DB, P, O_SH))
        bb = np.ascontiguousarray(
            np.broadcast_to(bias[o0 : o0 + O_SH].astype(np.float32), (P, O_SH))
        )
        bias_og.append(bb)

    in_maps = []
    for c in range(N_CORES):
        mg, og = c // OG, c % OG
        m0 = mg * M_SH
        xt = {}
        r4 = {}
        for name, arr in (("xt_hi", x_hi), ("xt_lo", x_lo)):
            r = arr[m0 : m0 + M_SH].reshape(MT, P, DB, P)  # [mt, m, db, d]
            r4[name] = np.ascontiguousarray(r.transpose(0, 3, 2, 1))  # [mt,d,db,m]
            xt[name] = r4[name].reshape(MT, P, DB * P)
        # startup-phase copy of m-tiles 0..1 in k-block-major order
        xt_pair = np.empty((DB, 2, 2, P, P), dtype=np.float16)
        for pi, name in enumerate(("xt_hi", "xt_lo")):
            xt_pair[:, :, pi] = r4[name][:2].transpose(2, 0, 1, 3)  # [db, mt, d, m]
        in_maps.append(
            {
                "xt_hi": xt["xt_hi"],
                "xt_lo": xt["xt_lo"],
                "xt_pair": xt_pair,
                "wt": wt_og[og],
                "biasb": bias_og[og],
            }
        )
    return in_maps


def run(inputs, trace=False):
    """Run the SPMD kernel; returns (full_output, BassKernelResults)."""
    if "nc" not in _CACHE:
        _CACHE["nc"] = _build()
    nc = _CACHE["nc"]
    in_maps = _prep_inputs(inputs["x"], inputs["weight"], inputs["bias"])
    res = run_bass_kernel_spmd(nc, in_maps, list(range(N_CORES)), trace=trace)
    out = np.empty((M_TOT, D_OUT), dtype=np.float32)
    for c in range(N_CORES):
        mg, og = c // OG, c % OG
        out[mg * M_SH : (mg + 1) * M_SH, og * O_SH : (og + 1) * O_SH] = res.results[
            c
        ]["out"]
    return out.reshape(B, S, D_OUT), res


def kernel(x, weight, bias):
    out, _ = run({"x": x, "weight": weight, "bias": bias})
    return out


# revision 26
# speedup vs baseline: 1.0106x; 1.0106x over previous
"""BitNetLinear on 8 Trainium2 NeuronCores.

Computes out = x @ sign(weight).T + bias for x[4,2048,4096] f32,
weight[4096,4096] f32, bias[4096] f32.

Strategy: 4-way data parallel over rows x 2-way tensor parallel over
out_features (each core owns a [2048, 2048] block of the [8192, 4096]
output; no collectives, host stitches blocks).

Per core the matmul runs in fp16 hi/lo split: x = hi + lo with both
halves fp16 (sign(weight) is exactly representable in fp16), both
passes accumulated into the same PSUM banks in fp32. This gives
~fp32 accuracy (measured max err / scale ~5e-7 vs float64) at bf16
matmul speed: 1 PE cycle per moving row vs 4 for true fp32.

Layouts are precomputed on the host so every DMA is contiguous:
  xt[mt, d, db*128+m] = x_pass[m0 + mt*128 + m, db*128 + d]
  wt[db, d, o]        = sign(weight)[o0 + o, db*128 + d]
The kernel keeps all 32 weight blocks resident in SBUF (128 KB per
partition), streams x tiles (one 1 MB contiguous DMA per pass per
m-tile), and accumulates each [128, 512] output chunk over 64 matmuls
(2 passes x 32 k-blocks) before a DVE eviction fused with the bias add.
"""

import sys
import types

import numpy as np

import concourse.mybir as mybir
import concourse.tile as tile
from concourse import bacc
from concourse.bass_utils import run_bass_kernel_spmd


def _ensure_axon_hooks():
    """run_bass_kernel_spmd(trace=True) (or BASS_TRACE=1 in the env) imports
    antenv.axon_hooks, which some agent images lack. Provide it, and register
    the ctypes NTFF hook if the boot shim is available, so tracing works (or
    degrades to a warning) instead of crashing."""
    try:
        import antenv.axon_hooks  # noqa: F401

        return
    except ImportError:
        pass
    m = types.ModuleType("antenv.axon_hooks")
    m._h = None
    m.set_axon_ntff_profile_hook = lambda h: setattr(m, "_h", h)
    m.get_axon_ntff_profile_hook = lambda: m._h
    sys.modules["antenv.axon_hooks"] = m
    try:
        import antenv

        antenv.axon_hooks = m
    except ImportError:
        pass
    try:
        from trn_agent_boot.trn_boot import _ntff_profile_via_ctypes

        m.set_axon_ntff_profile_hook(
            _ntff_profile_via_ctypes("/opt/axon/libaxon_pjrt.so")
        )
    except Exception:
        pass


_ensure_axon_hooks()

B, S, D_IN, D_OUT = 4, 2048, 4096, 4096
M_TOT = B * S  # 8192
N_CORES = 8
MG, OG = 4, 2  # data-parallel row groups x tensor-parallel out_feature groups
M_SH = M_TOT // MG  # 2048 rows per core
O_SH = D_OUT // OG  # 2048 out features per core
P = 128
DB = D_IN // P  # 32 contraction blocks
MT = M_SH // P  # 16 m-tiles per core
NF = 512  # moving free dim per matmul (one PSUM bank of fp32)
NCH = O_SH // NF  # 4 output chunks per m-tile

_CACHE = {}


def _build():
    nc = bacc.Bacc("TRN2", target_bir_lowering=False, debug=False)
    xt_hi_d = nc.dram_tensor(
        "xt_hi", [MT, P, DB * P], mybir.dt.float16, kind="ExternalInput"
    )
    xt_lo_d = nc.dram_tensor(
        "xt_lo", [MT, P, DB * P], mybir.dt.float16, kind="ExternalInput"
    )
    wt_d = nc.dram_tensor("wt", [DB, P, O_SH], mybir.dt.float16, kind="ExternalInput")
    # x for m-tiles 0..1 again, but in k-block-major layout for the startup
    # phase: per k-block one contiguous [128, 4*128] tile holding
    # (mt0-hi, mt0-lo, mt1-hi, mt1-lo) side by side in the free dim.
    xt_pair_d = nc.dram_tensor(
        "xt_pair", [DB, P, 4 * P], mybir.dt.float16, kind="ExternalInput"
    )
    bias_d = nc.dram_tensor("biasb", [P, O_SH], mybir.dt.float32, kind="ExternalInput")
    out_d = nc.dram_tensor("out", [M_SH, O_SH], mybir.dt.float32, kind="ExternalOutput")

    with tile.TileContext(nc) as tc:
        with (
            tc.tile_pool(name="wpool", bufs=1) as wpool,
            tc.tile_pool(name="xpool", bufs=2) as xpool,
            tc.tile_pool(name="psum", bufs=2, space="PSUM") as psum_pool,
        ):

            def load_x(mt):
                x_hi = xpool.tile(
                    [P, DB * P], mybir.dt.float16, name="x_hi", tag="xhi"
                )
                x_lo = xpool.tile(
                    [P, DB * P], mybir.dt.float16, name="x_lo", tag="xlo"
                )
                nc.sync.dma_start(out=x_hi[:], in_=xt_hi_d[mt])
                nc.sync.dma_start(out=x_lo[:], in_=xt_lo_d[mt])
                return x_hi, x_lo

            def alloc_psums(mt):
                return [
                    psum_pool.tile(
                        [P, NF], mybir.dt.float32, name=f"ps{oc}", tag=f"ps{oc}"
                    )
                    for oc in range(NCH)
                ]

            def mm_block(x_pair, psums, db, first, last):
                # 8 matmuls: hi+lo pass over one k-block into 4 psum banks
                for i, x_sb in enumerate(x_pair):
                    lhsT = x_sb[:, db * P : (db + 1) * P]
                    for oc in range(NCH):
                        nc.tensor.matmul(
                            psums[oc][:],
                            lhsT,
                            w_sb[db][:, oc * NF : (oc + 1) * NF],
                            start=first and i == 0,
                            stop=last and i == 1,
                        )

            def evict(opool, mt, psums):
                for oc in range(NCH):
                    o_sb = opool.tile(
                        [P, NF], mybir.dt.float32, name="o_sb", tag=f"o{oc}"
                    )
                    nc.vector.tensor_add(
                        o_sb[:], psums[oc][:], bias_sb[:, oc * NF : (oc + 1) * NF]
                    )
                    nc.sync.dma_start(
                        out=out_d[mt * P : (mt + 1) * P, oc * NF : (oc + 1) * NF],
                        in_=o_sb[:],
                    )

            w_sb = []
            with tc.tile_pool(name="xstart", bufs=1) as xstart_pool:
                # Startup phase x: per-k-block [128, 128] tiles so the first
                # matmul only waits for ~1.1 MB (w0 + 4 small x blocks), not
                # whole 2 MB x tiles behind the weight stream.
                xs_sb = {}
                for db in range(DB):
                    t = xstart_pool.tile(
                        [P, 4 * P],
                        mybir.dt.float16,
                        name=f"xs{db}",
                        tag=f"xs{db}",
                    )
                    nc.sync.dma_start(out=t[:], in_=xt_pair_d[db])
                    xs_sb[db] = t
                    # interleave weight blocks with startup-x so w[db] arrives
                    # roughly when the PE needs it
                    w = wpool.tile(
                        [P, O_SH], mybir.dt.float16, name=f"w{db}", tag=f"w{db}"
                    )
                    if db == 0:
                        # split w0 into partition chunks (each still fully
                        # contiguous in DRAM) so the eight DMA queues finish
                        # the first block together and the PE starts early
                        for j in range(4):
                            nc.sync.dma_start(
                                out=w[j * 32 : (j + 1) * 32, :],
                                in_=wt_d[db, j * 32 : (j + 1) * 32, :],
                            )
                    else:
                        nc.sync.dma_start(out=w[:], in_=wt_d[db])
                    w_sb.append(w)
                bias_sb = wpool.tile([P, O_SH], mybir.dt.float32, name="bias_sb")
                nc.sync.dma_start(out=bias_sb[:], in_=bias_d[:])

                # Prefetch x for m-tiles 2..3 now: their dma_starts must sit
                # ahead of the pair-phase eviction DMAs in the in-order sync
                # stream, else they head-of-line block until the pair phase
                # fully drains.
                x_next = {mt: load_x(mt) for mt in (2, 3)}

                # Startup phase: m-tiles 0+1 jointly, k-block-major, so PE
                # consumption (~3.4us per k-block) paces the weight stream
                # (~3.1us per block) instead of stalling for the full preload.
                psums01 = [alloc_psums(0), alloc_psums(1)]
                for db in range(DB):
                    for mt in range(2):
                        for pi in range(2):
                            g = mt * 2 + pi
                            lhsT = xs_sb[db][:, g * P : (g + 1) * P]
                            for oc in range(NCH):
                                nc.tensor.matmul(
                                    psums01[mt][oc][:],
                                    lhsT,
                                    w_sb[db][:, oc * NF : (oc + 1) * NF],
                                    start=db == 0 and pi == 0,
                                    stop=db == DB - 1 and pi == 1,
                                )
            # opool created only after xstart is released so their SBUF
            # address ranges can overlap in time.
            with tc.tile_pool(name="opool", bufs=2) as opool:
                for mt in range(2):
                    evict(opool, mt, psums01[mt])

                # Steady state: one m-tile at a time, double-buffered.
                for mt in range(2, MT):
                    x_pair = x_next.pop(mt) if mt in x_next else load_x(mt)
                    psums = alloc_psums(mt)
                    if mt < MT - 1:
                        for db in range(DB):
                            mm_block(x_pair, psums, db, db == 0, db == DB - 1)
                        evict(opool, mt, psums)
                    else:
                        # Last m-tile: oc-major so each output chunk's 64-matmul
                        # accumulation finishes (and evicts) as early as
                        # possible instead of all four at the very end.
                        for oc in range(NCH):
                            for db in range(DB):
                                for i, x_sb in enumerate(x_pair):
                                    nc.tensor.matmul(
                                        psums[oc][:],
                                        x_sb[:, db * P : (db + 1) * P],
                                        w_sb[db][:, oc * NF : (oc + 1) * NF],
                                        start=db == 0 and i == 0,
                                        stop=db == DB - 1 and i == 1,
                                    )
                            o_sb = opool.tile(
                                [P, NF], mybir.dt.float32, name="o_sb", tag=f"o{oc}"
                            )
                            nc.vector.tensor_add(
                                o_sb[:],
                                psums[oc][:],
                                bias_sb[:, oc * NF : (oc + 1) * NF],
                            )
                            nc.sync.dma_start(
                                out=out_d[
                                    mt * P : (mt + 1) * P, oc * NF : (oc + 1) * NF
                                ],
                                in_=o_sb[:],
                            )
    nc.compile()
    return nc


def _prep_inputs(x, weight, bias):
    x = np.asarray(x, dtype=np.float32)
    weight = np.asarray(weight, dtype=np.float32)
    bias = np.asarray(bias, dtype=np.float32)
    xf = np.ascontiguousarray(x.reshape(M_TOT, D_IN), dtype=np.float32)
    x_hi = xf.astype(np.float16)
    x_lo = (xf - x_hi.astype(np.float32)).astype(np.float16)

    qw = np.sign(weight.astype(np.float32)).astype(np.float16)  # [o, d]

    # per o-group weight block + broadcast bias, shared by all cores in group
    wt_og, bias_og = [], []
    for og in range(OG):
        o0 = og * O_SH
        blk = np.ascontiguousarray(qw[o0 : o0 + O_SH, :].T)  # [d, o]
        wt_og.append(blk.reshape(DB, P, O_SH))
        bb = np.ascontiguousarray(
            np.broadcast_to(bias[o0 : o0 + O_SH].astype(np.float32), (P, O_SH))
        )
        bias_og.append(bb)

    in_maps = []
    for c in range(N_CORES):
        mg, og = c // OG, c % OG
        m0 = mg * M_SH
        xt = {}
        r4 = {}
        for name, arr in (("xt_hi", x_hi), ("xt_lo", x_lo)):
            r = arr[m0 : m0 + M_SH].reshape(MT, P, DB, P)  # [mt, m, db, d]
            r4[name] = np.ascontiguousarray(r.transpose(0, 3, 2, 1))  # [mt,d,db,m]
            xt[name] = r4[name].reshape(MT, P, DB * P)
        # startup-phase copy of m-tiles 0..1 in k-block-major order:
        # xt_pair[db, d, (2*mt+pi)*128 + m]
        xt_pair = np.empty((DB, P, 4 * P), dtype=np.float16)
        for pi, name in enumerate(("xt_hi", "xt_lo")):
            blk = r4[name][:2].transpose(2, 0, 1, 3)  # [db, mt, d, m]
            for mt in range(2):
                g = mt * 2 + pi
                xt_pair[:, :, g * P : (g + 1) * P] = blk[:, mt]
        in_maps.append(
            {
                "xt_hi": xt["xt_hi"],
                "xt_lo": xt["xt_lo"],
                "xt_pair": xt_pair,
                "wt": wt_og[og],
                "biasb": bias_og[og],
            }
        )
    return in_maps


def run(inputs, trace=False):
    """Run the SPMD kernel; returns (full_output, BassKernelResults)."""
    if "nc" not in _CACHE:
        _CACHE["nc"] = _build()
    nc = _CACHE["nc"]
    in_maps = _prep_inputs(inputs["x"], inputs["weight"], inputs["bias"])
    res = run_bass_kernel_spmd(nc, in_maps, list(range(N_CORES)), trace=trace)
    out = np.empty((M_TOT, D_OUT), dtype=np.float32)
    for c in range(N_CORES):
        mg, og = c // OG, c % OG
        out[mg * M_SH : (mg + 1) * M_SH, og * O_SH : (og + 1) * O_SH] = res.results[
            c
        ]["out"]
    return out.reshape(B, S, D_OUT), res


def kernel(x, weight, bias):
    out, _ = run({"x": x, "weight": weight, "bias": bias})
    return out


# revision 28
# speedup vs baseline: 1.3237x; 1.3098x over previous
"""BitNetLinear on 8 Trainium2 NeuronCores.

Computes out = x @ sign(weight).T + bias for x[4,2048,4096] f32,
weight[4096,4096] f32, bias[4096] f32.

Strategy: 2-way data parallel over rows x 4-way tensor parallel over
out_features (each core owns a [4096, 1024] block of the [8192, 4096]
output; no collectives, host stitches blocks).

Per core the matmul splits x = hi + lo:
  hi = fp8e4m3(x), run with perf_mode=DoubleRow (2 fp8 MACs/cell/cycle,
       k=256 per matmul) -- HW-measured 1.84x fp16 throughput;
  lo = fp16(x - hi), run as normal fp16 matmuls (1 cycle/row).
sign(weight) is exact in both fp8e4m3 and fp16. Both passes accumulate
into the same PSUM banks in fp32. Measured accuracy vs float64:
rel-l2 ~5e-6 (vs 4.4e-7 for an fp16 hi/lo split, 1.3e-4 absmax for
numpy's own fp32 matmul), at ~1.3x the speed.

Layouts are precomputed on the host so every DMA is contiguous. Both
weight copies stay resident in SBUF (w16 64KB + w8 32KB per partition),
x tiles stream per m-tile, and each [128, 512] output chunk accumulates
16 DoubleRow + 32 fp16 matmuls before a DVE eviction fused with the
bias add. The first three m-tiles run jointly, k-block-major, so PE
consumption paces the 12 MB weight preload instead of stalling on it.
"""

import sys
import types

import numpy as np

import concourse.mybir as mybir
import concourse.tile as tile
from concourse import bacc
from concourse.bass_utils import run_bass_kernel_spmd


def _ensure_axon_hooks():
    """run_bass_kernel_spmd(trace=True) (or BASS_TRACE=1 in the env) imports
    antenv.axon_hooks, which some agent images lack. Provide it, and register
    the ctypes NTFF hook if the boot shim is available, so tracing works (or
    degrades to a warning) instead of crashing."""
    try:
        import antenv.axon_hooks  # noqa: F401

        return
    except ImportError:
        pass
    m = types.ModuleType("antenv.axon_hooks")
    m._h = None
    m.set_axon_ntff_profile_hook = lambda h: setattr(m, "_h", h)
    m.get_axon_ntff_profile_hook = lambda: m._h
    sys.modules["antenv.axon_hooks"] = m
    try:
        import antenv

        antenv.axon_hooks = m
    except ImportError:
        pass
    try:
        from trn_agent_boot.trn_boot import _ntff_profile_via_ctypes

        m.set_axon_ntff_profile_hook(
            _ntff_profile_via_ctypes("/opt/axon/libaxon_pjrt.so")
        )
    except Exception:
        pass


_ensure_axon_hooks()

B, S, D_IN, D_OUT = 4, 2048, 4096, 4096
M_TOT = B * S  # 8192
N_CORES = 8
MG, OG = 2, 4  # data-parallel row groups x tensor-parallel out_feature groups
M_SH = M_TOT // MG  # 4096 rows per core
O_SH = D_OUT // OG  # 1024 out features per core
P = 128
DB = D_IN // P  # 32 contraction blocks of 128
DP = DB // 2  # 16 contraction pairs of 256 (DoubleRow)
MT = M_SH // P  # 32 m-tiles per core
NF = 512  # moving free dim per matmul (one PSUM bank of fp32)
NCH = O_SH // NF  # 2 output chunks per m-tile
ST = 3  # m-tiles processed jointly in the startup phase

_CACHE = {}


def _build():
    nc = bacc.Bacc("TRN2", target_bir_lowering=False, debug=False)
    f8, f16, f32 = mybir.dt.float8e4, mybir.dt.float16, mybir.dt.float32

    # steady-state x, one m-tile per row: free = dp*256 + h*128 + m (hi8)
    # and db*128 + m (lo16)
    xh_d = nc.dram_tensor("xh", [MT, P, DB * P], f8, kind="ExternalInput")
    xl_d = nc.dram_tensor("xl", [MT, P, DB * P], f16, kind="ExternalInput")
    # startup copies of m-tiles 0..ST-1, k-block-major: per dp one contiguous
    # block per dtype covering the ST m-tiles
    xhs_d = nc.dram_tensor("xhs", [DP, P, ST * 2 * P], f8, kind="ExternalInput")
    xls_d = nc.dram_tensor("xls", [DP, P, ST * 2 * P], f16, kind="ExternalInput")
    # weights: fp8 pair layout per dp, fp16 per db
    w8_d = nc.dram_tensor("w8", [DP, P, 2 * O_SH], f8, kind="ExternalInput")
    w16_d = nc.dram_tensor("w16", [DB, P, O_SH], f16, kind="ExternalInput")
    bias_d = nc.dram_tensor("biasb", [P, O_SH], f32, kind="ExternalInput")
    out_d = nc.dram_tensor("out", [M_SH, O_SH], f32, kind="ExternalOutput")

    with tile.TileContext(nc) as tc:
        with (
            tc.tile_pool(name="wpool", bufs=1) as wpool,
            tc.tile_pool(name="xpool", bufs=2) as xpool,
            tc.tile_pool(name="psum", bufs=ST, space="PSUM") as psum_pool,
        ):

            def load_x(mt):
                x_hi = xpool.tile([P, DB * P], f8, name="x_hi", tag="xhi")
                x_lo = xpool.tile([P, DB * P], f16, name="x_lo", tag="xlo")
                nc.sync.dma_start(out=x_hi[:], in_=xh_d[mt])
                nc.sync.dma_start(out=x_lo[:], in_=xl_d[mt])
                return x_hi, x_lo

            def alloc_psums():
                return [
                    psum_pool.tile([P, NF], f32, name=f"ps{oc}", tag=f"ps{oc}")
                    for oc in range(NCH)
                ]

            def mm_hi(psums, lhsT3, dp, first):
                # lhsT3: [P, 2, 128] fp8 pair view; one DoubleRow MM per chunk
                for oc in range(NCH):
                    nc.tensor.matmul(
                        psums[oc][:],
                        lhsT3,
                        w8_sb[dp][:]
                        .rearrange("p (h o) -> p h o", h=2)[
                            :, :, oc * NF : (oc + 1) * NF
                        ],
                        start=first,
                        stop=False,
                        perf_mode=mybir.MatmulPerfMode.DoubleRow,
                    )

            def mm_lo(psums, lhsT, db, last):
                for oc in range(NCH):
                    nc.tensor.matmul(
                        psums[oc][:],
                        lhsT,
                        w16_sb[db][:, oc * NF : (oc + 1) * NF],
                        start=False,
                        stop=last,
                    )

            def mm_tile(x_pair, psums):
                # one m-tile: per k-pair, hi DoubleRow then the two lo blocks
                x_hi, x_lo = x_pair
                for dp in range(DP):
                    lhsT3 = x_hi[:, dp * 2 * P : (dp + 1) * 2 * P].rearrange(
                        "p (h m) -> p h m", h=2
                    )
                    mm_hi(psums, lhsT3, dp, dp == 0)
                    for j in range(2):
                        db = 2 * dp + j
                        mm_lo(
                            psums,
                            x_lo[:, db * P : (db + 1) * P],
                            db,
                            dp == DP - 1 and j == 1,
                        )

            def evict(opool, mt, psums, ocs=None):
                for oc in ocs if ocs is not None else range(NCH):
                    o_sb = opool.tile([P, NF], f32, name="o_sb", tag=f"o{oc}")
                    nc.vector.tensor_add(
                        o_sb[:], psums[oc][:], bias_sb[:, oc * NF : (oc + 1) * NF]
                    )
                    nc.sync.dma_start(
                        out=out_d[mt * P : (mt + 1) * P, oc * NF : (oc + 1) * NF],
                        in_=o_sb[:],
                    )

            w8_sb = []
            w16_sb = []
            with tc.tile_pool(name="xstart", bufs=1) as xstart_pool:
                # startup x (m-tiles 0..ST-1) in k-major order plus the
                # weight stream, interleaved so w[dp] lands as the PE needs it
                xhs_sb, xls_sb = [], []
                for dp in range(DP):
                    th = xstart_pool.tile(
                        [P, ST * 2 * P], f8, name=f"xhs{dp}", tag=f"xhs{dp}"
                    )
                    nc.sync.dma_start(out=th[:], in_=xhs_d[dp])
                    xhs_sb.append(th)
                    tl = xstart_pool.tile(
                        [P, ST * 2 * P], f16, name=f"xls{dp}", tag=f"xls{dp}"
                    )
                    nc.sync.dma_start(out=tl[:], in_=xls_d[dp])
                    xls_sb.append(tl)
                    w8 = wpool.tile(
                        [P, 2 * O_SH], f8, name=f"w8_{dp}", tag=f"w8_{dp}"
                    )
                    nc.sync.dma_start(out=w8[:], in_=w8_d[dp])
                    w8_sb.append(w8)
                    for j in range(2):
                        db = 2 * dp + j
                        w16 = wpool.tile(
                            [P, O_SH], f16, name=f"w16_{db}", tag=f"w16_{db}"
                        )
                        nc.sync.dma_start(out=w16[:], in_=w16_d[db])
                        w16_sb.append(w16)
                bias_sb = wpool.tile([P, O_SH], f32, name="bias_sb")
                nc.sync.dma_start(out=bias_sb[:], in_=bias_d[:])

                # prefetch steady-state x ahead of the startup evictions
                # (in-order sync stream: later dma_starts would head-of-line
                # block behind eviction DMAs otherwise)
                x_next = {mt: load_x(mt) for mt in (ST, ST + 1)}

                # startup: ST m-tiles jointly, k-major, paced by the weight
                # stream
                psums_st = [alloc_psums() for _ in range(ST)]
                for dp in range(DP):
                    for st in range(ST):
                        xh = xhs_sb[dp][
                            :, st * 2 * P : (st + 1) * 2 * P
                        ].rearrange("p (h m) -> p h m", h=2)
                        mm_hi(psums_st[st], xh, dp, dp == 0)
                        for j in range(2):
                            db = 2 * dp + j
                            mm_lo(
                                psums_st[st],
                                xls_sb[dp][
                                    :,
                                    (st * 2 + j) * P : (st * 2 + j + 1) * P,
                                ],
                                db,
                                dp == DP - 1 and j == 1,
                            )

            with tc.tile_pool(name="opool", bufs=2) as opool:
                for st in range(ST):
                    evict(opool, st, psums_st[st])

                for mt in range(ST, MT):
                    x_pair = x_next.pop(mt) if mt in x_next else load_x(mt)
                    psums = alloc_psums()
                    if mt < MT - 1:
                        mm_tile(x_pair, psums)
                        evict(opool, mt, psums)
                    else:
                        # last m-tile: oc-major so each output chunk finishes
                        # and evicts as early as possible
                        x_hi, x_lo = x_pair
                        for oc in range(NCH):
                            for dp in range(DP):
                                nc.tensor.matmul(
                                    psums[oc][:],
                                    x_hi[
                                        :, dp * 2 * P : (dp + 1) * 2 * P
                                    ].rearrange("p (h m) -> p h m", h=2),
                                    w8_sb[dp][:]
                                    .rearrange("p (h o) -> p h o", h=2)[
                                        :, :, oc * NF : (oc + 1) * NF
                                    ],
                                    start=dp == 0,
                                    stop=False,
                                    perf_mode=mybir.MatmulPerfMode.DoubleRow,
                                )
                            for db in range(DB):
                                nc.tensor.matmul(
                                    psums[oc][:],
                                    x_lo[:, db * P : (db + 1) * P],
                                    w16_sb[db][:, oc * NF : (oc + 1) * NF],
                                    start=False,
                                    stop=db == DB - 1,
                                )
                            evict(opool, mt, psums, ocs=[oc])
    nc.compile()
    return nc


def _prep_inputs(x, weight, bias):
    import ml_dtypes

    f8 = ml_dtypes.float8_e4m3
    x = np.asarray(x, dtype=np.float32)
    weight = np.asarray(weight, dtype=np.float32)
    bias = np.asarray(bias, dtype=np.float32)

    xf = np.ascontiguousarray(x.reshape(M_TOT, D_IN))
    x_hi = xf.astype(f8)
    x_lo = (xf - x_hi.astype(np.float32)).astype(np.float16)

    qw = np.sign(weight)  # [o, d] f32

    # per o-group weights + broadcast bias, shared by cores in the group
    w8_og, w16_og, bias_og = [], [], []
    for og in range(OG):
        o0 = og * O_SH
        blk = np.ascontiguousarray(qw[o0 : o0 + O_SH, :].T)  # [d, o] f32
        # w16[db, d_in, o]
        w16_og.append(blk.astype(np.float16).reshape(DB, P, O_SH))
        # w8[dp, d_in, h*O_SH + o]
        w8 = (
            blk.astype(f8)
            .reshape(DP, 2, P, O_SH)
            .transpose(0, 2, 1, 3)
            .reshape(DP, P, 2 * O_SH)
        )
        w8_og.append(np.ascontiguousarray(w8))
        bias_og.append(
            np.ascontiguousarray(
                np.broadcast_to(bias[o0 : o0 + O_SH], (P, O_SH))
            )
        )

    # per m-group x layouts, shared by cores in the group
    xh_mg, xl_mg, xhs_mg, xls_mg = [], [], [], []
    for mg in range(MG):
        m0 = mg * M_SH
        # hi8 steady state: [mt, d, dp*256 + h*128 + m]
        r = x_hi[m0 : m0 + M_SH].reshape(MT, P, DP, 2, P)  # [mt,m,dp,h,d]
        xh = np.ascontiguousarray(r.transpose(0, 4, 2, 3, 1)).reshape(
            MT, P, DB * P
        )
        xh_mg.append(xh)
        # lo16 steady state: [mt, d, db*128 + m]
        r = x_lo[m0 : m0 + M_SH].reshape(MT, P, DB, P)  # [mt,m,db,d]
        xl = np.ascontiguousarray(r.transpose(0, 3, 2, 1)).reshape(
            MT, P, DB * P
        )
        xl_mg.append(xl)
        # startup copies, k-major over the first ST m-tiles
        xhs = np.empty((DP, P, ST * 2 * P), dtype=f8)
        xls = np.empty((DP, P, ST * 2 * P), dtype=np.float16)
        for st in range(ST):
            xhs[:, :, st * 2 * P : (st + 1) * 2 * P] = (
                xh[st].reshape(P, DP, 2 * P).transpose(1, 0, 2)
            )
            xls[:, :, st * 2 * P : (st + 1) * 2 * P] = (
                xl[st].reshape(P, DP, 2 * P).transpose(1, 0, 2)
            )
        xhs_mg.append(xhs)
        xls_mg.append(xls)

    in_maps = []
    for c in range(N_CORES):
        mg, og = c // OG, c % OG
        in_maps.append(
            {
                "xh": xh_mg[mg],
                "xl": xl_mg[mg],
                "xhs": xhs_mg[mg],
                "xls": xls_mg[mg],
                "w8": w8_og[og],
                "w16": w16_og[og],
                "biasb": bias_og[og],
            }
        )
    return in_maps


def run(inputs, trace=False):
    """Run the SPMD kernel; returns (full_output, BassKernelResults)."""
    if "nc" not in _CACHE:
        _CACHE["nc"] = _build()
    nc = _CACHE["nc"]
    in_maps = _prep_inputs(inputs["x"], inputs["weight"], inputs["bias"])
    res = run_bass_kernel_spmd(nc, in_maps, list(range(N_CORES)), trace=trace)
    out = np.empty((M_TOT, D_OUT), dtype=np.float32)
    for c in range(N_CORES):
        mg, og = c // OG, c % OG
        out[mg * M_SH : (mg + 1) * M_SH, og * O_SH : (og + 1) * O_SH] = res.results[
            c
        ]["out"]
    return out.reshape(B, S, D_OUT), res


def kernel(x, weight, bias):
    out, _ = run({"x": x, "weight": weight, "bias": bias})
    return out


# revision 32
# speedup vs baseline: 1.3238x; 1.0001x over previous
"""BitNetLinear on 8 Trainium2 NeuronCores.

Computes out = x @ sign(weight).T + bias for x[4,2048,4096] f32,
weight[4096,4096] f32, bias[4096] f32.

Strategy: 2-way data parallel over rows x 4-way tensor parallel over
out_features (each core owns a [4096, 1024] block of the [8192, 4096]
output; no collectives, host stitches blocks).

Per core the matmul splits x = hi + lo:
  hi = fp8e4m3(x), run with perf_mode=DoubleRow (2 fp8 MACs/cell/cycle,
       k=256 per matmul) -- HW-measured 1.84x fp16 throughput;
  lo = fp16(x - hi), run as normal fp16 matmuls (1 cycle/row).
sign(weight) is exact in both fp8e4m3 and fp16. Both passes accumulate
into the same PSUM banks in fp32. Measured accuracy vs float64:
rel-l2 ~5e-6 (vs 4.4e-7 for an fp16 hi/lo split, 1.3e-4 absmax for
numpy's own fp32 matmul), at ~1.3x the speed.

Layouts are precomputed on the host so every DMA is contiguous. Both
weight copies stay resident in SBUF (w16 64KB + w8 32KB per partition),
x tiles stream per m-tile, and each [128, 512] output chunk accumulates
16 DoubleRow + 32 fp16 matmuls before a DVE eviction fused with the
bias add. The first three m-tiles run jointly, k-block-major, so PE
consumption paces the 12 MB weight preload instead of stalling on it.
"""

import sys
import types

import numpy as np

import concourse.mybir as mybir
import concourse.tile as tile
from concourse import bacc
from concourse.bass_utils import run_bass_kernel_spmd


def _ensure_axon_hooks():
    """run_bass_kernel_spmd(trace=True) (or BASS_TRACE=1 in the env) imports
    antenv.axon_hooks, which some agent images lack. Provide it, and register
    the ctypes NTFF hook if the boot shim is available, so tracing works (or
    degrades to a warning) instead of crashing."""
    try:
        import antenv.axon_hooks  # noqa: F401

        return
    except ImportError:
        pass
    m = types.ModuleType("antenv.axon_hooks")
    m._h = None
    m.set_axon_ntff_profile_hook = lambda h: setattr(m, "_h", h)
    m.get_axon_ntff_profile_hook = lambda: m._h
    sys.modules["antenv.axon_hooks"] = m
    try:
        import antenv

        antenv.axon_hooks = m
    except ImportError:
        pass
    try:
        from trn_agent_boot.trn_boot import _ntff_profile_via_ctypes

        m.set_axon_ntff_profile_hook(
            _ntff_profile_via_ctypes("/opt/axon/libaxon_pjrt.so")
        )
    except Exception:
        pass


_ensure_axon_hooks()

B, S, D_IN, D_OUT = 4, 2048, 4096, 4096
M_TOT = B * S  # 8192
N_CORES = 8
MG, OG = 2, 4  # data-parallel row groups x tensor-parallel out_feature groups
M_SH = M_TOT // MG  # 4096 rows per core
O_SH = D_OUT // OG  # 1024 out features per core
P = 128
DB = D_IN // P  # 32 contraction blocks of 128
DP = DB // 2  # 16 contraction pairs of 256 (DoubleRow)
MT = M_SH // P  # 32 m-tiles per core
NF = 512  # moving free dim per matmul (one PSUM bank of fp32)
NCH = O_SH // NF  # 2 output chunks per m-tile
ST = 3  # m-tiles processed jointly in the startup phase

_CACHE = {}


def _build():
    nc = bacc.Bacc("TRN2", target_bir_lowering=False, debug=False)
    f8, f16, f32 = mybir.dt.float8e4, mybir.dt.float16, mybir.dt.float32

    # steady-state x, one m-tile per row: free = dp*256 + h*128 + m (hi8)
    # and db*128 + m (lo16)
    xh_d = nc.dram_tensor("xh", [MT, P, DB * P], f8, kind="ExternalInput")
    xl_d = nc.dram_tensor("xl", [MT, P, DB * P], f16, kind="ExternalInput")
    # startup copies of m-tiles 0..ST-1, k-block-major: per dp one contiguous
    # block per dtype covering the ST m-tiles
    xhs_d = nc.dram_tensor("xhs", [DP, P, ST * 2 * P], f8, kind="ExternalInput")
    xls_d = nc.dram_tensor("xls", [DP, P, ST * 2 * P], f16, kind="ExternalInput")
    # weights: fp8 pair layout per dp, fp16 per db
    w8_d = nc.dram_tensor("w8", [DP, P, 2 * O_SH], f8, kind="ExternalInput")
    w16_d = nc.dram_tensor("w16", [DB, P, O_SH], f16, kind="ExternalInput")
    bias_d = nc.dram_tensor("biasb", [P, O_SH], f32, kind="ExternalInput")
    out_d = nc.dram_tensor("out", [M_SH, O_SH], f32, kind="ExternalOutput")

    with tile.TileContext(nc) as tc:
        with (
            tc.tile_pool(name="wpool", bufs=1) as wpool,
            tc.tile_pool(name="xpool", bufs=2) as xpool,
            tc.tile_pool(name="psum", bufs=ST, space="PSUM") as psum_pool,
        ):

            def load_x(mt):
                x_hi = xpool.tile([P, DB * P], f8, name="x_hi", tag="xhi")
                x_lo = xpool.tile([P, DB * P], f16, name="x_lo", tag="xlo")
                nc.sync.dma_start(out=x_hi[:], in_=xh_d[mt])
                nc.sync.dma_start(out=x_lo[:], in_=xl_d[mt])
                return x_hi, x_lo

            def alloc_psums():
                return [
                    psum_pool.tile([P, NF], f32, name=f"ps{oc}", tag=f"ps{oc}")
                    for oc in range(NCH)
                ]

            def mm_hi(psums, lhsT3, dp, last):
                # lhsT3: [P, 2, 128] fp8 pair view; one DoubleRow MM per chunk
                for oc in range(NCH):
                    nc.tensor.matmul(
                        psums[oc][:],
                        lhsT3,
                        w8_sb[dp][:]
                        .rearrange("p (h o) -> p h o", h=2)[
                            :, :, oc * NF : (oc + 1) * NF
                        ],
                        start=False,
                        stop=last,
                        perf_mode=mybir.MatmulPerfMode.DoubleRow,
                    )

            def mm_lo(psums, lhsT, db, first):
                for oc in range(NCH):
                    nc.tensor.matmul(
                        psums[oc][:],
                        lhsT,
                        w16_sb[db][:, oc * NF : (oc + 1) * NF],
                        start=first,
                        stop=False,
                    )

            def mm_tile(x_pair, psums):
                # one m-tile: per k-pair, the two fp16 lo blocks then the hi
                # DoubleRow MM -- the long 256-col fp8 LDWEIGHTS overlaps the
                # four preceding lo matmuls
                x_hi, x_lo = x_pair
                for dp in range(DP):
                    for j in range(2):
                        db = 2 * dp + j
                        mm_lo(
                            psums,
                            x_lo[:, db * P : (db + 1) * P],
                            db,
                            dp == 0 and j == 0,
                        )
                    lhsT3 = x_hi[:, dp * 2 * P : (dp + 1) * 2 * P].rearrange(
                        "p (h m) -> p h m", h=2
                    )
                    mm_hi(psums, lhsT3, dp, dp == DP - 1)

            def evict(opool, mt, psums, ocs=None):
                for oc in ocs if ocs is not None else range(NCH):
                    o_sb = opool.tile([P, NF], f32, name="o_sb", tag=f"o{oc}")
                    nc.vector.tensor_add(
                        o_sb[:], psums[oc][:], bias_sb[:, oc * NF : (oc + 1) * NF]
                    )
                    nc.sync.dma_start(
                        out=out_d[mt * P : (mt + 1) * P, oc * NF : (oc + 1) * NF],
                        in_=o_sb[:],
                    )

            w8_sb = []
            w16_sb = []
            with tc.tile_pool(name="xstart", bufs=1) as xstart_pool:
                # startup x (m-tiles 0..ST-1) in k-major order plus the
                # weight stream, interleaved so w[dp] lands as the PE needs it
                xhs_sb, xls_sb = [], []
                for dp in range(DP):
                    # issue in consumption order: lo x + lo weights first,
                    # then the hi (DoubleRow) pair
                    tl = xstart_pool.tile(
                        [P, ST * 2 * P], f16, name=f"xls{dp}", tag=f"xls{dp}"
                    )
                    nc.sync.dma_start(out=tl[:], in_=xls_d[dp])
                    xls_sb.append(tl)
                    for j in range(2):
                        db = 2 * dp + j
                        w16 = wpool.tile(
                            [P, O_SH], f16, name=f"w16_{db}", tag=f"w16_{db}"
                        )
                        nc.sync.dma_start(out=w16[:], in_=w16_d[db])
                        w16_sb.append(w16)
                    th = xstart_pool.tile(
                        [P, ST * 2 * P], f8, name=f"xhs{dp}", tag=f"xhs{dp}"
                    )
                    nc.sync.dma_start(out=th[:], in_=xhs_d[dp])
                    xhs_sb.append(th)
                    w8 = wpool.tile(
                        [P, 2 * O_SH], f8, name=f"w8_{dp}", tag=f"w8_{dp}"
                    )
                    nc.sync.dma_start(out=w8[:], in_=w8_d[dp])
                    w8_sb.append(w8)
                bias_sb = wpool.tile([P, O_SH], f32, name="bias_sb")
                nc.sync.dma_start(out=bias_sb[:], in_=bias_d[:])

                # prefetch steady-state x ahead of the startup evictions
                # (in-order sync stream: later dma_starts would head-of-line
                # block behind eviction DMAs otherwise)
                x_next = {mt: load_x(mt) for mt in (ST, ST + 1)}

                # startup: ST m-tiles jointly, k-major, paced by the weight
                # stream
                psums_st = [alloc_psums() for _ in range(ST)]
                for dp in range(DP):
                    for st in range(ST):
                        for j in range(2):
                            db = 2 * dp + j
                            mm_lo(
                                psums_st[st],
                                xls_sb[dp][
                                    :,
                                    (st * 2 + j) * P : (st * 2 + j + 1) * P,
                                ],
                                db,
                                dp == 0 and j == 0,
                            )
                        xh = xhs_sb[dp][
                            :, st * 2 * P : (st + 1) * 2 * P
                        ].rearrange("p (h m) -> p h m", h=2)
                        mm_hi(psums_st[st], xh, dp, dp == DP - 1)

            with tc.tile_pool(name="opool", bufs=2) as opool:
                for st in range(ST):
                    evict(opool, st, psums_st[st])

                for mt in range(ST, MT):
                    x_pair = x_next.pop(mt) if mt in x_next else load_x(mt)
                    psums = alloc_psums()
                    if mt < MT - 1:
                        mm_tile(x_pair, psums)
                        evict(opool, mt, psums)
                    else:
                        # last m-tile: oc-major so each output chunk finishes
                        # and evicts as early as possible
                        x_hi, x_lo = x_pair
                        for oc in range(NCH):
                            for db in range(DB):
                                nc.tensor.matmul(
                                    psums[oc][:],
                                    x_lo[:, db * P : (db + 1) * P],
                                    w16_sb[db][:, oc * NF : (oc + 1) * NF],
                                    start=db == 0,
                                    stop=False,
                                )
                            for dp in range(DP):
                                nc.tensor.matmul(
                                    psums[oc][:],
                                    x_hi[
                                        :, dp * 2 * P : (dp + 1) * 2 * P
                                    ].rearrange("p (h m) -> p h m", h=2),
                                    w8_sb[dp][:]
                                    .rearrange("p (h o) -> p h o", h=2)[
                                        :, :, oc * NF : (oc + 1) * NF
                                    ],
                                    start=False,
                                    stop=dp == DP - 1,
                                    perf_mode=mybir.MatmulPerfMode.DoubleRow,
                                )
                            evict(opool, mt, psums, ocs=[oc])
    nc.compile()
    return nc


def _prep_inputs(x, weight, bias):
    import ml_dtypes

    f8 = ml_dtypes.float8_e4m3
    x = np.asarray(x, dtype=np.float32)
    weight = np.asarray(weight, dtype=np.float32)
    bias = np.asarray(bias, dtype=np.float32)

    xf = np.ascontiguousarray(x.reshape(M_TOT, D_IN))
    x_hi = xf.astype(f8)
    x_lo = (xf - x_hi.astype(np.float32)).astype(np.float16)

    qw = np.sign(weight)  # [o, d] f32

    # per o-group weights + broadcast bias, shared by cores in the group
    w8_og, w16_og, bias_og = [], [], []
    for og in range(OG):
        o0 = og * O_SH
        blk = np.ascontiguousarray(qw[o0 : o0 + O_SH, :].T)  # [d, o] f32
        # w16[db, d_in, o]
        w16_og.append(blk.astype(np.float16).reshape(DB, P, O_SH))
        # w8[dp, d_in, h*O_SH + o]
        w8 = (
            blk.astype(f8)
            .reshape(DP, 2, P, O_SH)
            .transpose(0, 2, 1, 3)
            .reshape(DP, P, 2 * O_SH)
        )
        w8_og.append(np.ascontiguousarray(w8))
        bias_og.append(
            np.ascontiguousarray(
                np.broadcast_to(bias[o0 : o0 + O_SH], (P, O_SH))
            )
        )

    # per m-group x layouts, shared by cores in the group
    xh_mg, xl_mg, xhs_mg, xls_mg = [], [], [], []
    for mg in range(MG):
        m0 = mg * M_SH
        # hi8 steady state: [mt, d, dp*256 + h*128 + m]
        r = x_hi[m0 : m0 + M_SH].reshape(MT, P, DP, 2, P)  # [mt,m,dp,h,d]
        xh = np.ascontiguousarray(r.transpose(0, 4, 2, 3, 1)).reshape(
            MT, P, DB * P
        )
        xh_mg.append(xh)
        # lo16 steady state: [mt, d, db*128 + m]
        r = x_lo[m0 : m0 + M_SH].reshape(MT, P, DB, P)  # [mt,m,db,d]
        xl = np.ascontiguousarray(r.transpose(0, 3, 2, 1)).reshape(
            MT, P, DB * P
        )
        xl_mg.append(xl)
        # startup copies, k-major over the first ST m-tiles
        xhs = np.empty((DP, P, ST * 2 * P), dtype=f8)
        xls = np.empty((DP, P, ST * 2 * P), dtype=np.float16)
        for st in range(ST):
            xhs[:, :, st * 2 * P : (st + 1) * 2 * P] = (
                xh[st].reshape(P, DP, 2 * P).transpose(1, 0, 2)
            )
            xls[:, :, st * 2 * P : (st + 1) * 2 * P] = (
                xl[st].reshape(P, DP, 2 * P).transpose(1, 0, 2)
            )
        xhs_mg.append(xhs)
        xls_mg.append(xls)

    in_maps = []
    for c in range(N_CORES):
        mg, og = c // OG, c % OG
        in_maps.append(
            {
                "xh": xh_mg[mg],
                "xl": xl_mg[mg],
                "xhs": xhs_mg[mg],
                "xls": xls_mg[mg],
                "w8": w8_og[og],
                "w16": w16_og[og],
                "biasb": bias_og[og],
            }
        )
    return in_maps


def run(inputs, trace=False):
    """Run the SPMD kernel; returns (full_output, BassKernelResults)."""
    if "nc" not in _CACHE:
        _CACHE["nc"] = _build()
    nc = _CACHE["nc"]
    in_maps = _prep_inputs(inputs["x"], inputs["weight"], inputs["bias"])
    res = run_bass_kernel_spmd(nc, in_maps, list(range(N_CORES)), trace=trace)
    out = np.empty((M_TOT, D_OUT), dtype=np.float32)
    for c in range(N_CORES):
        mg, og = c // OG, c % OG
        out[mg * M_SH : (mg + 1) * M_SH, og * O_SH : (og + 1) * O_SH] = res.results[
            c
        ]["out"]
    return out.reshape(B, S, D_OUT), res


def kernel(x, weight, bias):
    out, _ = run({"x": x, "weight": weight, "bias": bias})
    return out


# revision 33
# speedup vs baseline: 1.3346x; 1.0081x over previous
"""BitNetLinear on 8 Trainium2 NeuronCores.

Computes out = x @ sign(weight).T + bias for x[4,2048,4096] f32,
weight[4096,4096] f32, bias[4096] f32.

Strategy: 2-way data parallel over rows x 4-way tensor parallel over
out_features (each core owns a [4096, 1024] block of the [8192, 4096]
output; no collectives, host stitches blocks).

Per core the matmul splits x = hi + lo:
  hi = fp8e4m3(x), run with perf_mode=DoubleRow (2 fp8 MACs/cell/cycle,
       k=256 per matmul) -- HW-measured 1.84x fp16 throughput;
  lo = fp16(x - hi), run as normal fp16 matmuls (1 cycle/row).
sign(weight) is exact in both fp8e4m3 and fp16. Both passes accumulate
into the same PSUM banks in fp32. Measured accuracy vs float64:
rel-l2 ~5e-6 (vs 4.4e-7 for an fp16 hi/lo split, 1.3e-4 absmax for
numpy's own fp32 matmul), at ~1.3x the speed.

Layouts are precomputed on the host so every DMA is contiguous. Both
weight copies stay resident in SBUF (w16 64KB + w8 32KB per partition),
x tiles stream per m-tile, and each [128, 512] output chunk accumulates
16 DoubleRow + 32 fp16 matmuls before a DVE eviction fused with the
bias add. The first three m-tiles run jointly, k-block-major, so PE
consumption paces the 12 MB weight preload instead of stalling on it.
"""

import sys
import types

import numpy as np

import concourse.mybir as mybir
import concourse.tile as tile
from concourse import bacc
from concourse.bass_utils import run_bass_kernel_spmd


def _ensure_axon_hooks():
    """run_bass_kernel_spmd(trace=True) (or BASS_TRACE=1 in the env) imports
    antenv.axon_hooks, which some agent images lack. Provide it, and register
    the ctypes NTFF hook if the boot shim is available, so tracing works (or
    degrades to a warning) instead of crashing."""
    try:
        import antenv.axon_hooks  # noqa: F401

        return
    except ImportError:
        pass
    m = types.ModuleType("antenv.axon_hooks")
    m._h = None
    m.set_axon_ntff_profile_hook = lambda h: setattr(m, "_h", h)
    m.get_axon_ntff_profile_hook = lambda: m._h
    sys.modules["antenv.axon_hooks"] = m
    try:
        import antenv

        antenv.axon_hooks = m
    except ImportError:
        pass
    try:
        from trn_agent_boot.trn_boot import _ntff_profile_via_ctypes

        m.set_axon_ntff_profile_hook(
            _ntff_profile_via_ctypes("/opt/axon/libaxon_pjrt.so")
        )
    except Exception:
        pass


_ensure_axon_hooks()

B, S, D_IN, D_OUT = 4, 2048, 4096, 4096
M_TOT = B * S  # 8192
N_CORES = 8
MG, OG = 2, 4  # data-parallel row groups x tensor-parallel out_feature groups
M_SH = M_TOT // MG  # 4096 rows per core
O_SH = D_OUT // OG  # 1024 out features per core
P = 128
DB = D_IN // P  # 32 contraction blocks of 128
DP = DB // 2  # 16 contraction pairs of 256 (DoubleRow)
MT = M_SH // P  # 32 m-tiles per core
NF = 512  # moving free dim per matmul (one PSUM bank of fp32)
NCH = O_SH // NF  # 2 output chunks per m-tile
ST = 3  # m-tiles processed jointly in the startup phase

_CACHE = {}


def _build():
    nc = bacc.Bacc("TRN2", target_bir_lowering=False, debug=False)
    f8, f16, f32 = mybir.dt.float8e4, mybir.dt.float16, mybir.dt.float32

    # steady-state x, one m-tile per row: free = dp*256 + h*128 + m (hi8)
    # and db*128 + m (lo16)
    xh_d = nc.dram_tensor("xh", [MT, P, DB * P], f8, kind="ExternalInput")
    xl_d = nc.dram_tensor("xl", [MT, P, DB * P], f16, kind="ExternalInput")
    # startup copies of m-tiles 0..ST-1, k-block-major: per dp one contiguous
    # block per dtype covering the ST m-tiles
    xhs_d = nc.dram_tensor("xhs", [DP, P, ST * 2 * P], f8, kind="ExternalInput")
    xls_d = nc.dram_tensor("xls", [DP, P, ST * 2 * P], f16, kind="ExternalInput")
    # weights: fp8 pair layout per dp, fp16 per db
    w8_d = nc.dram_tensor("w8", [DP, P, 2 * O_SH], f8, kind="ExternalInput")
    w16_d = nc.dram_tensor("w16", [DB, P, O_SH], f16, kind="ExternalInput")
    bias_d = nc.dram_tensor("biasb", [P, O_SH], f32, kind="ExternalInput")
    out_d = nc.dram_tensor("out", [M_SH, O_SH], f32, kind="ExternalOutput")

    with tile.TileContext(nc) as tc:
        with (
            tc.tile_pool(name="wpool", bufs=1) as wpool,
            tc.tile_pool(name="xpool", bufs=2) as xpool,
            tc.tile_pool(name="psum", bufs=ST, space="PSUM") as psum_pool,
        ):

            def load_x(mt):
                x_hi = xpool.tile([P, DB * P], f8, name="x_hi", tag="xhi")
                x_lo = xpool.tile([P, DB * P], f16, name="x_lo", tag="xlo")
                nc.sync.dma_start(out=x_hi[:], in_=xh_d[mt])
                nc.sync.dma_start(out=x_lo[:], in_=xl_d[mt])
                return x_hi, x_lo

            def alloc_psums():
                return [
                    psum_pool.tile([P, NF], f32, name=f"ps{oc}", tag=f"ps{oc}")
                    for oc in range(NCH)
                ]

            def mm_hi(psums, lhsT3, dp, last):
                # lhsT3: [P, 2, 128] fp8 pair view; one DoubleRow MM per chunk
                for oc in range(NCH):
                    nc.tensor.matmul(
                        psums[oc][:],
                        lhsT3,
                        w8_sb[dp][:]
                        .rearrange("p (h o) -> p h o", h=2)[
                            :, :, oc * NF : (oc + 1) * NF
                        ],
                        start=False,
                        stop=last,
                        perf_mode=mybir.MatmulPerfMode.DoubleRow,
                    )

            def mm_lo(psums, lhsT, db, first):
                for oc in range(NCH):
                    nc.tensor.matmul(
                        psums[oc][:],
                        lhsT,
                        w16_sb[db][:, oc * NF : (oc + 1) * NF],
                        start=first,
                        stop=False,
                    )

            def mm_tile(x_pair, psums):
                # one m-tile: full fp16 lo block, then full DoubleRow hi block
                # -- only two fp16<->DoubleRow weight-path mode switches per
                # m-tile instead of one per k-pair
                x_hi, x_lo = x_pair
                for db in range(DB):
                    mm_lo(psums, x_lo[:, db * P : (db + 1) * P], db, db == 0)
                for dp in range(DP):
                    lhsT3 = x_hi[:, dp * 2 * P : (dp + 1) * 2 * P].rearrange(
                        "p (h m) -> p h m", h=2
                    )
                    mm_hi(psums, lhsT3, dp, dp == DP - 1)

            def evict(opool, mt, psums, ocs=None):
                for oc in ocs if ocs is not None else range(NCH):
                    o_sb = opool.tile([P, NF], f32, name="o_sb", tag=f"o{oc}")
                    nc.vector.tensor_add(
                        o_sb[:], psums[oc][:], bias_sb[:, oc * NF : (oc + 1) * NF]
                    )
                    nc.sync.dma_start(
                        out=out_d[mt * P : (mt + 1) * P, oc * NF : (oc + 1) * NF],
                        in_=o_sb[:],
                    )

            w8_sb = []
            w16_sb = []
            with tc.tile_pool(name="xstart", bufs=1) as xstart_pool:
                # startup x (m-tiles 0..ST-1) in k-major order plus the
                # weight stream, interleaved so w[dp] lands as the PE needs it
                xhs_sb, xls_sb = [], []
                for dp in range(DP):
                    # issue in consumption order: lo x + lo weights first,
                    # then the hi (DoubleRow) pair
                    tl = xstart_pool.tile(
                        [P, ST * 2 * P], f16, name=f"xls{dp}", tag=f"xls{dp}"
                    )
                    nc.sync.dma_start(out=tl[:], in_=xls_d[dp])
                    xls_sb.append(tl)
                    for j in range(2):
                        db = 2 * dp + j
                        w16 = wpool.tile(
                            [P, O_SH], f16, name=f"w16_{db}", tag=f"w16_{db}"
                        )
                        nc.sync.dma_start(out=w16[:], in_=w16_d[db])
                        w16_sb.append(w16)
                    th = xstart_pool.tile(
                        [P, ST * 2 * P], f8, name=f"xhs{dp}", tag=f"xhs{dp}"
                    )
                    nc.sync.dma_start(out=th[:], in_=xhs_d[dp])
                    xhs_sb.append(th)
                    w8 = wpool.tile(
                        [P, 2 * O_SH], f8, name=f"w8_{dp}", tag=f"w8_{dp}"
                    )
                    nc.sync.dma_start(out=w8[:], in_=w8_d[dp])
                    w8_sb.append(w8)
                bias_sb = wpool.tile([P, O_SH], f32, name="bias_sb")
                nc.sync.dma_start(out=bias_sb[:], in_=bias_d[:])

                # prefetch steady-state x ahead of the startup evictions
                # (in-order sync stream: later dma_starts would head-of-line
                # block behind eviction DMAs otherwise)
                x_next = {mt: load_x(mt) for mt in (ST, ST + 1)}

                # startup: ST m-tiles jointly, k-major, paced by the weight
                # stream
                psums_st = [alloc_psums() for _ in range(ST)]
                for dp in range(DP):
                    for st in range(ST):
                        for j in range(2):
                            db = 2 * dp + j
                            mm_lo(
                                psums_st[st],
                                xls_sb[dp][
                                    :,
                                    (st * 2 + j) * P : (st * 2 + j + 1) * P,
                                ],
                                db,
                                dp == 0 and j == 0,
                            )
                        xh = xhs_sb[dp][
                            :, st * 2 * P : (st + 1) * 2 * P
                        ].rearrange("p (h m) -> p h m", h=2)
                        mm_hi(psums_st[st], xh, dp, dp == DP - 1)

            with tc.tile_pool(name="opool", bufs=2) as opool:
                for st in range(ST):
                    evict(opool, st, psums_st[st])

                for mt in range(ST, MT):
                    x_pair = x_next.pop(mt) if mt in x_next else load_x(mt)
                    psums = alloc_psums()
                    if mt < MT - 1:
                        mm_tile(x_pair, psums)
                        evict(opool, mt, psums)
                    else:
                        # last m-tile: oc-major so each output chunk finishes
                        # and evicts as early as possible
                        x_hi, x_lo = x_pair
                        for oc in range(NCH):
                            for db in range(DB):
                                nc.tensor.matmul(
                                    psums[oc][:],
                                    x_lo[:, db * P : (db + 1) * P],
                                    w16_sb[db][:, oc * NF : (oc + 1) * NF],
                                    start=db == 0,
                                    stop=False,
                                )
                            for dp in range(DP):
                                nc.tensor.matmul(
                                    psums[oc][:],
                                    x_hi[
                                        :, dp * 2 * P : (dp + 1) * 2 * P
                                    ].rearrange("p (h m) -> p h m", h=2),
                                    w8_sb[dp][:]
                                    .rearrange("p (h o) -> p h o", h=2)[
                                        :, :, oc * NF : (oc + 1) * NF
                                    ],
                                    start=False,
                                    stop=dp == DP - 1,
                                    perf_mode=mybir.MatmulPerfMode.DoubleRow,
                                )
                            evict(opool, mt, psums, ocs=[oc])
    nc.compile()
    return nc


def _prep_inputs(x, weight, bias):
    import ml_dtypes

    f8 = ml_dtypes.float8_e4m3
    x = np.asarray(x, dtype=np.float32)
    weight = np.asarray(weight, dtype=np.float32)
    bias = np.asarray(bias, dtype=np.float32)

    xf = np.ascontiguousarray(x.reshape(M_TOT, D_IN))
    x_hi = xf.astype(f8)
    x_lo = (xf - x_hi.astype(np.float32)).astype(np.float16)

    qw = np.sign(weight)  # [o, d] f32

    # per o-group weights + broadcast bias, shared by cores in the group
    w8_og, w16_og, bias_og = [], [], []
    for og in range(OG):
        o0 = og * O_SH
        blk = np.ascontiguousarray(qw[o0 : o0 + O_SH, :].T)  # [d, o] f32
        # w16[db, d_in, o]
        w16_og.append(blk.astype(np.float16).reshape(DB, P, O_SH))
        # w8[dp, d_in, h*O_SH + o]
        w8 = (
            blk.astype(f8)
            .reshape(DP, 2, P, O_SH)
            .transpose(0, 2, 1, 3)
            .reshape(DP, P, 2 * O_SH)
        )
        w8_og.append(np.ascontiguousarray(w8))
        bias_og.append(
            np.ascontiguousarray(
                np.broadcast_to(bias[o0 : o0 + O_SH], (P, O_SH))
            )
        )

    # per m-group x layouts, shared by cores in the group
    xh_mg, xl_mg, xhs_mg, xls_mg = [], [], [], []
    for mg in range(MG):
        m0 = mg * M_SH
        # hi8 steady state: [mt, d, dp*256 + h*128 + m]
        r = x_hi[m0 : m0 + M_SH].reshape(MT, P, DP, 2, P)  # [mt,m,dp,h,d]
        xh = np.ascontiguousarray(r.transpose(0, 4, 2, 3, 1)).reshape(
            MT, P, DB * P
        )
        xh_mg.append(xh)
        # lo16 steady state: [mt, d, db*128 + m]
        r = x_lo[m0 : m0 + M_SH].reshape(MT, P, DB, P)  # [mt,m,db,d]
        xl = np.ascontiguousarray(r.transpose(0, 3, 2, 1)).reshape(
            MT, P, DB * P
        )
        xl_mg.append(xl)
        # startup copies, k-major over the first ST m-tiles
        xhs = np.empty((DP, P, ST * 2 * P), dtype=f8)
        xls = np.empty((DP, P, ST * 2 * P), dtype=np.float16)
        for st in range(ST):
            xhs[:, :, st * 2 * P : (st + 1) * 2 * P] = (
                xh[st].reshape(P, DP, 2 * P).transpose(1, 0, 2)
            )
            xls[:, :, st * 2 * P : (st + 1) * 2 * P] = (
                xl[st].reshape(P, DP, 2 * P).transpose(1, 0, 2)
            )
        xhs_mg.append(xhs)
        xls_mg.append(xls)

    in_maps = []
    for c in range(N_CORES):
        mg, og = c // OG, c % OG
        in_maps.append(
            {
                "xh": xh_mg[mg],
                "xl": xl_mg[mg],
                "xhs": xhs_mg[mg],
                "xls": xls_mg[mg],
                "w8": w8_og[og],
                "w16": w16_og[og],
                "biasb": bias_og[og],
            }
        )
    return in_maps


def run(inputs, trace=False):
    """Run the SPMD kernel; returns (full_output, BassKernelResults)."""
    if "nc" not in _CACHE:
        _CACHE["nc"] = _build()
    nc = _CACHE["nc"]
    in_maps = _prep_inputs(inputs["x"], inputs["weight"], inputs["bias"])
    res = run_bass_kernel_spmd(nc, in_maps, list(range(N_CORES)), trace=trace)
    out = np.empty((M_TOT, D_OUT), dtype=np.float32)
    for c in range(N_CORES):
        mg, og = c // OG, c % OG
        out[mg * M_SH : (mg + 1) * M_SH, og * O_SH : (og + 1) * O_SH] = res.results[
            c
        ]["out"]
    return out.reshape(B, S, D_OUT), res


def kernel(x, weight, bias):
    out, _ = run({"x": x, "weight": weight, "bias": bias})
    return out


# revision 35
# speedup vs baseline: 1.3363x; 1.0013x over previous
"""BitNetLinear on 8 Trainium2 NeuronCores.

Computes out = x @ sign(weight).T + bias for x[4,2048,4096] f32,
weight[4096,4096] f32, bias[4096] f32.

Strategy: 2-way data parallel over rows x 4-way tensor parallel over
out_features (each core owns a [4096, 1024] block of the [8192, 4096]
output; no collectives, host stitches blocks).

Per core the matmul splits x = hi + lo:
  hi = fp8e4m3(x), run with perf_mode=DoubleRow (2 fp8 MACs/cell/cycle,
       k=256 per matmul) -- HW-measured 1.84x fp16 throughput;
  lo = fp16(x - hi), run as normal fp16 matmuls (1 cycle/row).
sign(weight) is exact in both fp8e4m3 and fp16. Both passes accumulate
into the same PSUM banks in fp32. Measured accuracy vs float64:
rel-l2 ~5e-6 (vs 4.4e-7 for an fp16 hi/lo split, 1.3e-4 absmax for
numpy's own fp32 matmul), at ~1.3x the speed.

Layouts are precomputed on the host so every DMA is contiguous. Both
weight copies stay resident in SBUF (w16 64KB + w8 32KB per partition),
x tiles stream per m-tile, and each [128, 512] output chunk accumulates
16 DoubleRow + 32 fp16 matmuls before a DVE eviction fused with the
bias add. The first three m-tiles run jointly, k-block-major, so PE
consumption paces the 12 MB weight preload instead of stalling on it.
"""

import sys
import types

import numpy as np

import concourse.mybir as mybir
import concourse.tile as tile
from concourse import bacc
from concourse.bass_utils import run_bass_kernel_spmd


def _ensure_axon_hooks():
    """run_bass_kernel_spmd(trace=True) (or BASS_TRACE=1 in the env) imports
    antenv.axon_hooks, which some agent images lack. Provide it, and register
    the ctypes NTFF hook if the boot shim is available, so tracing works (or
    degrades to a warning) instead of crashing."""
    try:
        import antenv.axon_hooks  # noqa: F401

        return
    except ImportError:
        pass
    m = types.ModuleType("antenv.axon_hooks")
    m._h = None
    m.set_axon_ntff_profile_hook = lambda h: setattr(m, "_h", h)
    m.get_axon_ntff_profile_hook = lambda: m._h
    sys.modules["antenv.axon_hooks"] = m
    try:
        import antenv

        antenv.axon_hooks = m
    except ImportError:
        pass
    try:
        from trn_agent_boot.trn_boot import _ntff_profile_via_ctypes

        m.set_axon_ntff_profile_hook(
            _ntff_profile_via_ctypes("/opt/axon/libaxon_pjrt.so")
        )
    except Exception:
        pass


_ensure_axon_hooks()

B, S, D_IN, D_OUT = 4, 2048, 4096, 4096
M_TOT = B * S  # 8192
N_CORES = 8
MG, OG = 2, 4  # data-parallel row groups x tensor-parallel out_feature groups
M_SH = M_TOT // MG  # 4096 rows per core
O_SH = D_OUT // OG  # 1024 out features per core
P = 128
DB = D_IN // P  # 32 contraction blocks of 128
DP = DB // 2  # 16 contraction pairs of 256 (DoubleRow)
MT = M_SH // P  # 32 m-tiles per core
NF = 512  # moving free dim per matmul (one PSUM bank of fp32)
NCH = O_SH // NF  # 2 output chunks per m-tile
ST = 3  # m-tiles processed jointly in the startup phase

_CACHE = {}


def _build():
    nc = bacc.Bacc("TRN2", target_bir_lowering=False, debug=False)
    f8, f16, f32 = mybir.dt.float8e4, mybir.dt.float16, mybir.dt.float32

    # steady-state x, one m-tile per row: free = dp*256 + h*128 + m (hi8)
    # and db*128 + m (lo16)
    xh_d = nc.dram_tensor("xh", [MT, P, DB * P], f8, kind="ExternalInput")
    xl_d = nc.dram_tensor("xl", [MT, P, DB * P], f16, kind="ExternalInput")
    # startup copies of m-tiles 0..ST-1, k-block-major: per dp one contiguous
    # block per dtype covering the ST m-tiles
    xhs_d = nc.dram_tensor("xhs", [DP, P, ST * 2 * P], f8, kind="ExternalInput")
    xls_d = nc.dram_tensor("xls", [DP, P, ST * 2 * P], f16, kind="ExternalInput")
    # weights: fp8 pair layout per dp, fp16 per db
    w8_d = nc.dram_tensor("w8", [DP, P, 2 * O_SH], f8, kind="ExternalInput")
    w16_d = nc.dram_tensor("w16", [DB, P, O_SH], f16, kind="ExternalInput")
    bias_d = nc.dram_tensor("biasb", [P, O_SH], f32, kind="ExternalInput")
    out_d = nc.dram_tensor("out", [M_SH, O_SH], f32, kind="ExternalOutput")

    with tile.TileContext(nc) as tc:
        with (
            tc.tile_pool(name="wpool", bufs=1) as wpool,
            tc.tile_pool(name="xpool", bufs=2) as xpool,
            tc.tile_pool(name="psum", bufs=ST, space="PSUM") as psum_pool,
        ):

            def load_x(mt):
                x_hi = xpool.tile([P, DB * P], f8, name="x_hi", tag="xhi")
                x_lo = xpool.tile([P, DB * P], f16, name="x_lo", tag="xlo")
                nc.sync.dma_start(out=x_hi[:], in_=xh_d[mt])
                nc.sync.dma_start(out=x_lo[:], in_=xl_d[mt])
                return x_hi, x_lo

            def alloc_psums():
                return [
                    psum_pool.tile([P, NF], f32, name=f"ps{oc}", tag=f"ps{oc}")
                    for oc in range(NCH)
                ]

            def mm_hi(psums, lhsT3, dp, last):
                # lhsT3: [P, 2, 128] fp8 pair view; one DoubleRow MM per chunk
                for oc in range(NCH):
                    nc.tensor.matmul(
                        psums[oc][:],
                        lhsT3,
                        w8_sb[dp][:]
                        .rearrange("p (h o) -> p h o", h=2)[
                            :, :, oc * NF : (oc + 1) * NF
                        ],
                        start=False,
                        stop=last,
                        perf_mode=mybir.MatmulPerfMode.DoubleRow,
                    )

            def mm_lo(psums, lhsT, db, first):
                for oc in range(NCH):
                    nc.tensor.matmul(
                        psums[oc][:],
                        lhsT,
                        w16_sb[db][:, oc * NF : (oc + 1) * NF],
                        start=first,
                        stop=False,
                    )

            def mm_tile(x_pair, psums, lo_first=True):
                # one m-tile: full fp16 lo block and full DoubleRow hi block
                # as two clean runs (2 weight-path mode switches per m-tile,
                # not one per k-pair); callers alternate lo_first by m-tile
                # parity so tile boundaries are same-mode (1 switch/tile net)
                x_hi, x_lo = x_pair

                def lo_block(first):
                    for db in range(DB):
                        mm_lo(
                            psums,
                            x_lo[:, db * P : (db + 1) * P],
                            db,
                            first and db == 0,
                        )

                def hi_block(last):
                    for dp in range(DP):
                        lhsT3 = x_hi[
                            :, dp * 2 * P : (dp + 1) * 2 * P
                        ].rearrange("p (h m) -> p h m", h=2)
                        mm_hi(psums, lhsT3, dp, last and dp == DP - 1)

                if lo_first:
                    lo_block(True)
                    hi_block(True)
                else:
                    # start flag must be on the first issued matmul; swap the
                    # roles: hi block opens the accumulation group
                    for dp in range(DP):
                        lhsT3 = x_hi[
                            :, dp * 2 * P : (dp + 1) * 2 * P
                        ].rearrange("p (h m) -> p h m", h=2)
                        for oc in range(NCH):
                            nc.tensor.matmul(
                                psums[oc][:],
                                lhsT3,
                                w8_sb[dp][:]
                                .rearrange("p (h o) -> p h o", h=2)[
                                    :, :, oc * NF : (oc + 1) * NF
                                ],
                                start=dp == 0,
                                stop=False,
                                perf_mode=mybir.MatmulPerfMode.DoubleRow,
                            )
                    for db in range(DB):
                        for oc in range(NCH):
                            nc.tensor.matmul(
                                psums[oc][:],
                                x_lo[:, db * P : (db + 1) * P],
                                w16_sb[db][:, oc * NF : (oc + 1) * NF],
                                start=False,
                                stop=db == DB - 1,
                            )

            def evict(opool, mt, psums, ocs=None):
                for oc in ocs if ocs is not None else range(NCH):
                    o_sb = opool.tile([P, NF], f32, name="o_sb", tag=f"o{oc}")
                    nc.vector.tensor_add(
                        o_sb[:], psums[oc][:], bias_sb[:, oc * NF : (oc + 1) * NF]
                    )
                    nc.sync.dma_start(
                        out=out_d[mt * P : (mt + 1) * P, oc * NF : (oc + 1) * NF],
                        in_=o_sb[:],
                    )

            w8_sb = []
            w16_sb = []
            with tc.tile_pool(name="xstart", bufs=1) as xstart_pool:
                # startup x (m-tiles 0..ST-1) in k-major order plus the
                # weight stream, interleaved so w[dp] lands as the PE needs it
                xhs_sb, xls_sb = [], []
                for dp in range(DP):
                    # issue in consumption order: lo x + lo weights first,
                    # then the hi (DoubleRow) pair
                    tl = xstart_pool.tile(
                        [P, ST * 2 * P], f16, name=f"xls{dp}", tag=f"xls{dp}"
                    )
                    nc.sync.dma_start(out=tl[:], in_=xls_d[dp])
                    xls_sb.append(tl)
                    for j in range(2):
                        db = 2 * dp + j
                        w16 = wpool.tile(
                            [P, O_SH], f16, name=f"w16_{db}", tag=f"w16_{db}"
                        )
                        nc.sync.dma_start(out=w16[:], in_=w16_d[db])
                        w16_sb.append(w16)
                    th = xstart_pool.tile(
                        [P, ST * 2 * P], f8, name=f"xhs{dp}", tag=f"xhs{dp}"
                    )
                    nc.sync.dma_start(out=th[:], in_=xhs_d[dp])
                    xhs_sb.append(th)
                    w8 = wpool.tile(
                        [P, 2 * O_SH], f8, name=f"w8_{dp}", tag=f"w8_{dp}"
                    )
                    nc.sync.dma_start(out=w8[:], in_=w8_d[dp])
                    w8_sb.append(w8)
                bias_sb = wpool.tile([P, O_SH], f32, name="bias_sb")
                nc.sync.dma_start(out=bias_sb[:], in_=bias_d[:])

                # prefetch steady-state x ahead of the startup evictions
                # (in-order sync stream: later dma_starts would head-of-line
                # block behind eviction DMAs otherwise)
                x_next = {mt: load_x(mt) for mt in (ST, ST + 1)}

                # startup: ST m-tiles jointly, k-major, paced by the weight
                # stream
                psums_st = [alloc_psums() for _ in range(ST)]
                for dp in range(DP):
                    for st in range(ST):
                        for j in range(2):
                            db = 2 * dp + j
                            mm_lo(
                                psums_st[st],
                                xls_sb[dp][
                                    :,
                                    (st * 2 + j) * P : (st * 2 + j + 1) * P,
                                ],
                                db,
                                dp == 0 and j == 0,
                            )
                        xh = xhs_sb[dp][
                            :, st * 2 * P : (st + 1) * 2 * P
                        ].rearrange("p (h m) -> p h m", h=2)
                        mm_hi(psums_st[st], xh, dp, dp == DP - 1)

            with tc.tile_pool(name="opool", bufs=2) as opool:
                for st in range(ST):
                    evict(opool, st, psums_st[st])

                for mt in range(ST, MT):
                    x_pair = x_next.pop(mt) if mt in x_next else load_x(mt)
                    psums = alloc_psums()
                    if mt < MT - 1:
                        # startup phase ends on a hi (DoubleRow) matmul, so
                        # the first steady tile opens hi; alternate after
                        mm_tile(x_pair, psums, lo_first=(mt - ST) % 2 == 1)
                        evict(opool, mt, psums)
                    else:
                        # last m-tile: oc-major so each output chunk finishes
                        # and evicts as early as possible
                        x_hi, x_lo = x_pair
                        for oc in range(NCH):
                            for db in range(DB):
                                nc.tensor.matmul(
                                    psums[oc][:],
                                    x_lo[:, db * P : (db + 1) * P],
                                    w16_sb[db][:, oc * NF : (oc + 1) * NF],
                                    start=db == 0,
                                    stop=False,
                                )
                            for dp in range(DP):
                                nc.tensor.matmul(
                                    psums[oc][:],
                                    x_hi[
                                        :, dp * 2 * P : (dp + 1) * 2 * P
                                    ].rearrange("p (h m) -> p h m", h=2),
                                    w8_sb[dp][:]
                                    .rearrange("p (h o) -> p h o", h=2)[
                                        :, :, oc * NF : (oc + 1) * NF
                                    ],
                                    start=False,
                                    stop=dp == DP - 1,
                                    perf_mode=mybir.MatmulPerfMode.DoubleRow,
                                )
                            evict(opool, mt, psums, ocs=[oc])
    nc.compile()
    return nc


def _prep_inputs(x, weight, bias):
    import ml_dtypes

    f8 = ml_dtypes.float8_e4m3
    x = np.asarray(x, dtype=np.float32)
    weight = np.asarray(weight, dtype=np.float32)
    bias = np.asarray(bias, dtype=np.float32)

    xf = np.ascontiguousarray(x.reshape(M_TOT, D_IN))
    x_hi = xf.astype(f8)
    x_lo = (xf - x_hi.astype(np.float32)).astype(np.float16)

    qw = np.sign(weight)  # [o, d] f32

    # per o-group weights + broadcast bias, shared by cores in the group
    w8_og, w16_og, bias_og = [], [], []
    for og in range(OG):
        o0 = og * O_SH
        blk = np.ascontiguousarray(qw[o0 : o0 + O_SH, :].T)  # [d, o] f32
        # w16[db, d_in, o]
        w16_og.append(blk.astype(np.float16).reshape(DB, P, O_SH))
        # w8[dp, d_in, h*O_SH + o]
        w8 = (
            blk.astype(f8)
            .reshape(DP, 2, P, O_SH)
            .transpose(0, 2, 1, 3)
            .reshape(DP, P, 2 * O_SH)
        )
        w8_og.append(np.ascontiguousarray(w8))
        bias_og.append(
            np.ascontiguousarray(
                np.broadcast_to(bias[o0 : o0 + O_SH], (P, O_SH))
            )
        )

    # per m-group x layouts, shared by cores in the group
    xh_mg, xl_mg, xhs_mg, xls_mg = [], [], [], []
    for mg in range(MG):
        m0 = mg * M_SH
        # hi8 steady state: [mt, d, dp*256 + h*128 + m]
        r = x_hi[m0 : m0 + M_SH].reshape(MT, P, DP, 2, P)  # [mt,m,dp,h,d]
        xh = np.ascontiguousarray(r.transpose(0, 4, 2, 3, 1)).reshape(
            MT, P, DB * P
        )
        xh_mg.append(xh)
        # lo16 steady state: [mt, d, db*128 + m]
        r = x_lo[m0 : m0 + M_SH].reshape(MT, P, DB, P)  # [mt,m,db,d]
        xl = np.ascontiguousarray(r.transpose(0, 3, 2, 1)).reshape(
            MT, P, DB * P
        )
        xl_mg.append(xl)
        # startup copies, k-major over the first ST m-tiles
        xhs = np.empty((DP, P, ST * 2 * P), dtype=f8)
        xls = np.empty((DP, P, ST * 2 * P), dtype=np.float16)
        for st in range(ST):
            xhs[:, :, st * 2 * P : (st + 1) * 2 * P] = (
                xh[st].reshape(P, DP, 2 * P).transpose(1, 0, 2)
            )
            xls[:, :, st * 2 * P : (st + 1) * 2 * P] = (
                xl[st].reshape(P, DP, 2 * P).transpose(1, 0, 2)
            )
        xhs_mg.append(xhs)
        xls_mg.append(xls)

    in_maps = []
    for c in range(N_CORES):
        mg, og = c // OG, c % OG
        in_maps.append(
            {
                "xh": xh_mg[mg],
                "xl": xl_mg[mg],
                "xhs": xhs_mg[mg],
                "xls": xls_mg[mg],
                "w8": w8_og[og],
                "w16": w16_og[og],
                "biasb": bias_og[og],
            }
        )
    return in_maps


def run(inputs, trace=False):
    """Run the SPMD kernel; returns (full_output, BassKernelResults)."""
    if "nc" not in _CACHE:
        _CACHE["nc"] = _build()
    nc = _CACHE["nc"]
    in_maps = _prep_inputs(inputs["x"], inputs["weight"], inputs["bias"])
    res = run_bass_kernel_spmd(nc, in_maps, list(range(N_CORES)), trace=trace)
    out = np.empty((M_TOT, D_OUT), dtype=np.float32)
    for c in range(N_CORES):
        mg, og = c // OG, c % OG
        out[mg * M_SH : (mg + 1) * M_SH, og * O_SH : (og + 1) * O_SH] = res.results[
            c
        ]["out"]
    return out.reshape(B, S, D_OUT), res


def kernel(x, weight, bias):
    out, _ = run({"x": x, "weight": weight, "bias": bias})
    return out
